# revision 1
# baseline (speedup 1.0000x reference)
"""Trainium2 Bass kernel for nn_ExpertsChooseMaskedExpand (MoE routing).

Reference computes (per batch b):
    xd[e,c,j] = sum_t mask[t,e,c] * x[t,e,j]          (dispatch)
    y[e,c,o]  = sum_j xd[e,c,j] * w[e,o,j] + bias[o]  (expert GEMM)
    out[t,o]  = sum_{e,c} comb[t,e,c] * y[e,c,o]      (combine)

We use associativity to contract comb with xd first:
    z[t,e,j] = sum_c comb[t,e,c] * xd[e,c,j]
    out[t,o] = sum_{e,j} z[t,e,j] * w[e,o,j] + bias[o] * S[t],
    S[t] = sum_{e,c} comb[t,e,c]
which cuts FLOPs ~3.4x and never materializes y (B,E,C,O).

Sharding: 8 cores; core k handles batch b=k//2, token half h=k%2 (2048
tokens). Each core computes its batch's full xd locally (dispatch work
duplicated across the pair) so no cross-core communication is needed.

Phasing: the head phase (dispatch) is DMA-bound and the combine phase is
PE-bound, so the z-stage (B) is interleaved into the combine phase per
t-chunk: comb loads ride under combine matmuls instead of inflating the
head phase. All matmuls run in bf16 with fp32 PSUM accumulation; inputs
are cast and re-laid-out on the host so every DMA is wide and contiguous.
"""

import numpy as np
import ml_dtypes

BF16 = ml_dtypes.bfloat16

B, T, E, C = 4, 4096, 8, 512
I = 128            # per-expert input features
O = 4096           # out_features
NCORES = 8
TLOC = B * T // NCORES      # 2048 tokens per core
NTT = T // 128              # 32 token tiles per batch (dispatch)
NQ = 4                      # dispatch chunk groups (8 token-tiles each)
NCT = C // 128              # 4 c-tiles
NTC = TLOC // 512           # 4 t-chunks per core (z stage)
NOT = O // 512              # 8 o-tiles
NTTL = TLOC // 128          # 16 local token tiles (final stage)

_CACHE = {}


def _build():
    import concourse.bass as bass
    import concourse.tile as tile
    import concourse.bacc as bacc
    import concourse.mybir as mybir

    f32 = mybir.dt.float32
    bf16 = mybir.dt.bfloat16
    ts = bass.ts

    nc = bacc.Bacc(None, target_bir_lowering=False, debug=False)

    xh = nc.dram_tensor("xh", [E, 128, NTT, I], bf16, kind="ExternalInput")
    mh = nc.dram_tensor("mh", [E, 128, NTT, C], bf16, kind="ExternalInput")
    cbt = nc.dram_tensor("cbt", [E, NCT, 128, TLOC], bf16, kind="ExternalInput")
    wf = nc.dram_tensor("wf", [128, E, O], bf16, kind="ExternalInput")
    ident = nc.dram_tensor("ident", [128, 128], bf16, kind="ExternalInput")
    out_d = nc.dram_tensor("out", [TLOC, O], f32, kind="ExternalOutput")

    with tile.TileContext(nc) as tc:
        with (
            tc.tile_pool(name="persist", bufs=1) as persist,
            tc.tile_pool(name="psum", bufs=1, space="PSUM") as psum,
        ):
            wf_sb = persist.tile([128, E, O], bf16, tag="wf")
            id_sb = persist.tile([128, 128], bf16, tag="ident")
            nc.scalar.dma_start(id_sb[:], ident[:])

            xd = {}   # e -> xd tile [128c, (ct j)] bf16
            zt = {}   # (e, tc) -> z^T tile [128j, 512t] bf16

            def stage_b(e, tch, cb_pool, cb_bufs):
                # z^T[e][tch] = xd[e] (c x j) contracted with comb^T
                cb_t = cb_pool.tile([128, NCT, 512], bf16, tag="cb",
                                    bufs=cb_bufs, name=f"cb{e}_{tch}")
                for ct in range(NCT):
                    nc.sync.dma_start(cb_t[:, ct, :],
                                      cbt[e, ct, :, ts(tch, 512)])
                ps_b = psum.tile([128, 512], f32, tag="psB", bufs=2,
                                 name=f"psB{e}_{tch}")
                for ct in range(NCT):
                    nc.tensor.matmul(
                        ps_b[:],
                        xd[e][:, ts(ct, 128)],
                        cb_t[:, ct, :],
                        start=(ct == 0),
                        stop=(ct == NCT - 1),
                    )
                z_sb = persist.tile([128, 512], bf16, tag=f"zt{e}_{tch}",
                                    name=f"zt{e}_{tch}")
                nc.vector.tensor_copy(z_sb[:], ps_b[:])
                zt[(e, tch)] = z_sb

            # ---- Head phase: dispatch (DMA-bound) + B(tc=0) ----
            pout = {}  # (tt, ot) -> bf16 partial of combine over e=0..3 (tc0)
            pqueue = [(ptt, pot) for ptt in range(4) for pot in range(NOT)]

            with (
                tc.tile_pool(name="head", bufs=1) as head,
                tc.tile_pool(name="psumA", bufs=1, space="PSUM") as psum_a,
                tc.tile_pool(name="psumP", bufs=1, space="PSUM") as psum_p,
            ):
                for e in range(E):
                    # one PSUM bank per ct: start=True zeroes a whole 2KB
                    # zero region, so accumulation groups must not share one
                    ps_a = [psum_a.tile([128, 128], f32, tag="psA", bufs=4,
                                        name=f"psA{e}_{ct}") for ct in range(NCT)]
                    chunks = ([(0, 2), (2, 2), (4, 4)] if e == 0 else []) + \
                        [(q * 8, 8) for q in range(1 if e == 0 else 0, NQ)]
                    for q0, qn in chunks:
                        mh_t = head.tile([128, 8, C], bf16, tag="mh", bufs=5,
                                         name=f"mh{e}_{q0}")
                        nc.sync.dma_start(mh_t[:, 0:qn, :],
                                          mh[e, :, q0:q0 + qn, :])
                        xh_t = head.tile([128, 8, I], bf16, tag="xh", bufs=5,
                                         name=f"xh{e}_{q0}")
                        nc.scalar.dma_start(xh_t[:, 0:qn, :],
                                            xh[e, :, q0:q0 + qn, :])
                        for i in range(qn):
                            tt = q0 + i
                            for ct in range(NCT):
                                nc.tensor.matmul(
                                    ps_a[ct][:],
                                    mh_t[:, i, ts(ct, 128)],
                                    xh_t[:, i, :],
                                    start=(tt == 0),
                                    stop=(tt == NTT - 1),
                                )
                        if e >= E // 2 and pqueue:
                            for ptt, pot in [pqueue.pop(0) for _ in
                                             range(min(2, len(pqueue)))]:
                                ptc, pm = ptt // 4, ptt % 4
                                ps_p = psum_p.tile(
                                    [128, 512], f32, tag="psP", bufs=2,
                                    name=f"psP{ptt}_{pot}")
                                for pe in range(E // 2):
                                    nc.tensor.matmul(
                                        ps_p[:],
                                        zt[(pe, ptc)][:, ts(pm, 128)],
                                        wf_sb[:, pe, ts(pot, 512)],
                                        start=(pe == 0),
                                        stop=(pe == E // 2 - 1),
                                    )
                                po = persist.tile(
                                    [128, 512], bf16, tag=f"po{ptt}_{pot}",
                                    name=f"po{ptt}_{pot}")
                                nc.vector.tensor_copy(po[:], ps_p[:])
                                pout[(ptt, pot)] = po
                    # weight slice prefetch rides behind this expert's loads
                    nc.scalar.dma_start(wf_sb[:, e, :], wf[:, e, :])
                    xd_sb = persist.tile([128, C], bf16, tag="xd", bufs=8,
                                         name=f"xd{e}")
                    for ct in range(NCT):
                        nc.vector.tensor_copy(xd_sb[:, ts(ct, 128)], ps_a[ct][:])
                    xd[e] = xd_sb
                    stage_b(e, 0, head, 2)

            # ---- Combine phase (PE-bound), stage B interleaved per tc ----
            with (
                tc.tile_pool(name="tail", bufs=1) as tail,
                tc.tile_pool(name="psumC", bufs=1, space="PSUM") as psum_c,
            ):
                for tcg in range(NTC):
                    for tt in range(tcg * 4, tcg * 4 + 4):
                        m = tt % 4
                        out_sb = tail.tile([128, O // 2], f32, tag="out",
                                           bufs=4, name=f"out{tt}")
                        for ot in range(NOT):
                            if ot == NOT // 2:
                                nc.scalar.dma_start(
                                    out_d[ts(tt, 128), 0:O // 2], out_sb[:])
                                out_sb = tail.tile([128, O // 2], f32,
                                                   tag="out", bufs=4,
                                                   name=f"out{tt}b")
                            ps_c = psum_c.tile([128, 512], f32, tag="psC",
                                               bufs=6, name=f"psC{tt}_{ot}")
                            e0 = E // 2 if tcg == 0 else 0
                            for e in range(e0, E):
                                nc.tensor.matmul(
                                    ps_c[:],
                                    zt[(e, tcg)][:, ts(m, 128)],
                                    wf_sb[:, e, ts(ot, 512)],
                                    start=(e == e0),
                                    stop=(tcg != 0 and e == E - 1),
                                )
                            if tcg == 0:
                                nc.tensor.matmul(
                                    ps_c[:], id_sb[:], pout[(tt, ot)][:],
                                    start=False, stop=True)
                            nc.vector.tensor_copy(
                                out_sb[:, ts(ot % 4, 512)], ps_c[:])
                        nc.scalar.dma_start(
                            out_d[ts(tt, 128), O // 2:O], out_sb[:])
                    if tcg + 1 < NTC:
                        for e in range(E):
                            stage_b(e, tcg + 1, tail, 6)

    nc.compile()
    return nc


def _prep_inputs(x, weight, bias, combine_array, dispatch_mask):
    """Host-side cast to bf16 + re-layout for contiguous device DMA."""
    x = np.asarray(x, np.float32)
    weight = np.asarray(weight, np.float32)
    bias = np.asarray(bias, np.float32)
    comb = np.asarray(combine_array, np.float32)
    mask = np.asarray(dispatch_mask, np.float32)

    # xh[b]: (E, 128, NTT, I); xh[b][e, p, tt, j] = x[b, tt*128+p, e, j]
    xh = np.ascontiguousarray(
        x.reshape(B, NTT, 128, E, I).transpose(0, 3, 2, 1, 4)).astype(BF16)
    # mh[b]: (E, 128, NTT, C)
    mh = np.ascontiguousarray(
        mask.reshape(B, NTT, 128, E, C).transpose(0, 3, 2, 1, 4)).astype(BF16)
    # cbt[b][h]: (E, NCT, 128, TLOC);
    # [..., e, ct, p, t] = comb[b, h*TLOC+t, e, ct*128+p]
    cbt = np.ascontiguousarray(
        comb.reshape(B, 2, TLOC, E, NCT, 128).transpose(0, 1, 3, 4, 5, 2)
    ).astype(BF16)
    # wf: (128, E, O); wf[j, e, o] = weight.reshape(E, O, I)[e, o, j]
    wf = np.ascontiguousarray(
        weight.reshape(E, O, I).transpose(2, 0, 1)).astype(BF16)
    # S[b, t] = sum_{e,c} comb[b, t, e, c] -- bias*S added on host in f32
    s = comb.sum(axis=(2, 3))
    idm = np.eye(128, dtype=BF16)

    in_maps = []
    for k in range(NCORES):
        b, h = k // 2, k % 2
        in_maps.append({
            "xh": xh[b], "mh": mh[b], "cbt": cbt[b, h], "wf": wf,
            "ident": idm,
        })
    return in_maps, s, bias


def kernel(x, weight, bias, combine_array, dispatch_mask):
    from concourse import bass_utils

    if "nc" not in _CACHE:
        _CACHE["nc"] = _build()
    nc = _CACHE["nc"]

    in_maps, s, bias_f = _prep_inputs(
        x, weight, bias, combine_array, dispatch_mask)
    res = bass_utils.run_bass_kernel_spmd(
        nc, in_maps, core_ids=list(range(NCORES)))
    out = np.stack([res.results[k]["out"] for k in range(NCORES)])
    out = out.reshape(B, T, O)
    out += s[:, :, None] * bias_f[None, None, :]
    return out.astype(np.float32)



# revision 2
# speedup vs baseline: 1.1921x; 1.1921x over previous
"""Trainium2 Bass kernel for nn_ExpertsChooseMaskedExpand (MoE routing).

Reference computes (per batch b):
    xd[e,c,j] = sum_t mask[t,e,c] * x[t,e,j]          (dispatch)
    y[e,c,o]  = sum_j xd[e,c,j] * w[e,o,j] + bias[o]  (expert GEMM)
    out[t,o]  = sum_{e,c} comb[t,e,c] * y[e,c,o]      (combine)

We use associativity to contract comb with xd first:
    z[t,e,j] = sum_c comb[t,e,c] * xd[e,c,j]
    out[t,o] = sum_{e,j} z[t,e,j] * w[e,o,j] + bias[o] * S[t],
    S[t] = sum_{e,c} comb[t,e,c]
which cuts FLOPs ~3.4x and never materializes y (B,E,C,O).

Sharding: 8 cores; core k handles batch b=k//2 and expert group
h=k%2 (experts h*4..h*4+4) over ALL 4096 tokens. Each core produces a
partial out (T, O) summed over its 4 experts only; the host adds the
two partials of each batch pair (plus bias*S). This halves both the
dispatch-mask DMA and the dispatch matmul work per core versus
splitting tokens (where dispatch must be duplicated across the pair).

Dispatch runs xh-stationary: one 128x128 ldweights per token tile and
a 512-wide mask stream, producing xd^T[j,c] in PSUM; 16 PE transposes
recover xd[c,j] for the z stage. All matmuls are bf16 with fp32 PSUM
accumulation; partial outputs are stored bf16 (host sums in fp32).
"""

import numpy as np
import ml_dtypes

BF16 = ml_dtypes.bfloat16

B, T, E, C = 4, 4096, 8, 512
I = 128            # per-expert input features
O = 4096           # out_features
NCORES = 8
EL = E // 2        # experts per core
NTT = T // 128     # 32 token tiles
NCT = C // 128     # 4 c-tiles
NTCH = T // 512    # 8 t-chunks (z / combine granularity)
NOT = O // 512     # 8 o-tiles

_CACHE = {}


def _build():
    import concourse.bass as bass
    import concourse.tile as tile
    import concourse.bacc as bacc
    import concourse.mybir as mybir

    f32 = mybir.dt.float32
    bf16 = mybir.dt.bfloat16
    ts = bass.ts

    nc = bacc.Bacc(None, target_bir_lowering=False, debug=False)

    xh = nc.dram_tensor("xh", [EL, 128, NTT, I], bf16, kind="ExternalInput")
    mh = nc.dram_tensor("mh", [EL, 128, NTT, C], bf16, kind="ExternalInput")
    cbt = nc.dram_tensor("cbt", [EL, NCT, 128, T], bf16, kind="ExternalInput")
    wf = nc.dram_tensor("wf", [128, EL, O], bf16, kind="ExternalInput")
    ident = nc.dram_tensor("ident", [128, 128], bf16, kind="ExternalInput")
    out_d = nc.dram_tensor("out", [T, O], bf16, kind="ExternalOutput")

    with tile.TileContext(nc) as tc:
        with (
            tc.tile_pool(name="persist", bufs=1) as persist,
            tc.tile_pool(name="psumB", bufs=1, space="PSUM") as psumb,
        ):
            wf_sb = persist.tile([128, EL, O], bf16, tag="wf")
            id_sb = persist.tile([128, 128], bf16, tag="ident")
            nc.scalar.dma_start(id_sb[:], ident[:])

            xd = {}   # e -> xd tile [128c, (ct j)] bf16
            zt = {}   # (e, tch) -> z^T tile [128j, 512t] bf16

            def stage_b(e, tch, cb_pool, cb_bufs):
                # z^T[e][tch][j, t] = sum_c xd[e][c, j] * comb^T[c, t]
                cb_t = cb_pool.tile([128, NCT, 512], bf16, tag="cb",
                                    bufs=cb_bufs, name=f"cb{e}_{tch}")
                for ct in range(NCT):
                    nc.sync.dma_start(cb_t[:, ct, :],
                                      cbt[e, ct, :, ts(tch, 512)])
                ps_b = psumb.tile([128, 512], f32, tag="psB", bufs=2,
                                  name=f"psB{e}_{tch}")
                for ct in range(NCT):
                    nc.tensor.matmul(
                        ps_b[:],
                        xd[e][:, ts(ct, 128)],
                        cb_t[:, ct, :],
                        start=(ct == 0),
                        stop=(ct == NCT - 1),
                    )
                z_sb = persist.tile([128, 512], bf16, tag=f"zt{e}_{tch}",
                                    name=f"zt{e}_{tch}")
                nc.vector.tensor_copy(z_sb[:], ps_b[:])
                zt[(e, tch)] = z_sb

            # ---- Head phase: dispatch (DMA-bound on the mask stream) ----
            with (
                tc.tile_pool(name="head", bufs=1) as head,
                tc.tile_pool(name="psumD", bufs=1, space="PSUM") as psumd,
            ):
                for e in range(EL):
                    xh_t = head.tile([128, NTT, I], bf16, tag="xh", bufs=2,
                                     name=f"xh{e}")
                    nc.scalar.dma_start(xh_t[:], xh[e])
                    # xd^T accumulator: [128j, 512c], one chain over all tt
                    ps_d = psumd.tile([128, C], f32, tag="psD", bufs=2,
                                      name=f"psD{e}")
                    for q in range(NTT // 8):
                        mh_t = head.tile([128, 8, C], bf16, tag="mh", bufs=5,
                                         name=f"mh{e}_{q}")
                        nc.sync.dma_start(mh_t[:], mh[e, :, q * 8:q * 8 + 8, :])
                        for i in range(8):
                            tt = q * 8 + i
                            nc.tensor.matmul(
                                ps_d[:],
                                xh_t[:, tt, :],
                                mh_t[:, i, :],
                                start=(tt == 0),
                                stop=(tt == NTT - 1),
                            )
                    xdT_sb = head.tile([128, C], bf16, tag="xdT", bufs=2,
                                       name=f"xdT{e}")
                    nc.vector.tensor_copy(xdT_sb[:], ps_d[:])
                    xd_sb = persist.tile([128, C], bf16, tag=f"xd{e}",
                                         name=f"xd{e}")
                    for ct in range(NCT):
                        ps_t = psumd.tile([128, 128], bf16, tag="psT", bufs=2,
                                          name=f"psT{e}_{ct}")
                        nc.tensor.transpose(ps_t[:], xdT_sb[:, ts(ct, 128)],
                                            id_sb[:])
                        nc.scalar.copy(xd_sb[:, ts(ct, 128)], ps_t[:])
                    xd[e] = xd_sb
                    # weight slice prefetch rides behind this expert's loads
                    nc.scalar.dma_start(wf_sb[:, e, :], wf[:, e, :])
                    stage_b(e, 0, head, 2)

            # ---- Combine phase (PE-bound), stage B interleaved per tch ----
            with (
                tc.tile_pool(name="tail", bufs=1) as tail,
                tc.tile_pool(name="psumC", bufs=1, space="PSUM") as psumc,
            ):
                for tcg in range(NTCH):
                    for tt in range(tcg * 4, tcg * 4 + 4):
                        m = tt % 4
                        out_sb = tail.tile([128, O], bf16, tag="out",
                                           bufs=3, name=f"out{tt}")
                        for ot in range(NOT):
                            ps_c = psumc.tile([128, 512], f32, tag="psC",
                                              bufs=5, name=f"psC{tt}_{ot}")
                            for e in range(EL):
                                nc.tensor.matmul(
                                    ps_c[:],
                                    zt[(e, tcg)][:, ts(m, 128)],
                                    wf_sb[:, e, ts(ot, 512)],
                                    start=(e == 0),
                                    stop=(e == EL - 1),
                                )
                            if ot % 2 == 0:
                                nc.vector.tensor_copy(
                                    out_sb[:, ts(ot, 512)], ps_c[:])
                            else:
                                nc.scalar.copy(
                                    out_sb[:, ts(ot, 512)], ps_c[:])
                        nc.scalar.dma_start(out_d[ts(tt, 128), :], out_sb[:])
                    if tcg + 1 < NTCH:
                        for e in range(EL):
                            stage_b(e, tcg + 1, tail, 6)

    nc.compile()
    return nc


def _prep_inputs(x, weight, bias, combine_array, dispatch_mask):
    """Host-side cast to bf16 + re-layout for contiguous device DMA."""
    x = np.asarray(x, np.float32)
    weight = np.asarray(weight, np.float32)
    bias = np.asarray(bias, np.float32)
    comb = np.asarray(combine_array, np.float32)
    mask = np.asarray(dispatch_mask, np.float32)

    # xh: (B, E, 128, NTT, I); xh[b, e, p, tt, j] = x[b, tt*128+p, e, j]
    xh = np.ascontiguousarray(
        x.reshape(B, NTT, 128, E, I).transpose(0, 3, 2, 1, 4)).astype(BF16)
    # mh: (B, E, 128, NTT, C)
    mh = np.ascontiguousarray(
        mask.reshape(B, NTT, 128, E, C).transpose(0, 3, 2, 1, 4)).astype(BF16)
    # cbt: (B, E, NCT, 128, T); [..., e, ct, p, t] = comb[b, t, e, ct*128+p]
    cbt = np.ascontiguousarray(
        comb.reshape(B, T, E, NCT, 128).transpose(0, 2, 3, 4, 1)).astype(BF16)
    # wf: (128, E, O); wf[j, e, o] = weight.reshape(E, O, I)[e, o, j]
    wf = np.ascontiguousarray(
        weight.reshape(E, O, I).transpose(2, 0, 1)).astype(BF16)
    # S[b, t] = sum_{e,c} comb[b, t, e, c] -- bias*S added on host in f32
    s = comb.sum(axis=(2, 3))
    idm = np.eye(128, dtype=BF16)

    in_maps = []
    for k in range(NCORES):
        b, h = k // 2, k % 2
        es = slice(h * EL, (h + 1) * EL)
        in_maps.append({
            "xh": np.ascontiguousarray(xh[b, es]),
            "mh": np.ascontiguousarray(mh[b, es]),
            "cbt": np.ascontiguousarray(cbt[b, es]),
            "wf": np.ascontiguousarray(wf[:, es, :]),
            "ident": idm,
        })
    return in_maps, s, bias


def kernel(x, weight, bias, combine_array, dispatch_mask):
    from concourse import bass_utils

    if "nc" not in _CACHE:
        _CACHE["nc"] = _build()
    nc = _CACHE["nc"]

    in_maps, s, bias_f = _prep_inputs(
        x, weight, bias, combine_array, dispatch_mask)
    res = bass_utils.run_bass_kernel_spmd(
        nc, in_maps, core_ids=list(range(NCORES)))
    out = np.empty((B, T, O), np.float32)
    for b in range(B):
        out[b] = res.results[2 * b]["out"].astype(np.float32)
        out[b] += res.results[2 * b + 1]["out"].astype(np.float32)
    out += s[:, :, None] * bias_f[None, None, :]
    return out


# revision 9
# speedup vs baseline: 1.2309x; 1.0325x over previous
"""Trainium2 Bass kernel for nn_ExpertsChooseMaskedExpand (MoE routing).

Reference computes (per batch b):
    xd[e,c,j] = sum_t mask[t,e,c] * x[t,e,j]          (dispatch)
    y[e,c,o]  = sum_j xd[e,c,j] * w[e,o,j] + bias[o]  (expert GEMM)
    out[t,o]  = sum_{e,c} comb[t,e,c] * y[e,c,o]      (combine)

We use associativity to contract comb with xd first:
    z[t,e,j] = sum_c comb[t,e,c] * xd[e,c,j]
    out[t,o] = sum_{e,j} z[t,e,j] * w[e,o,j] + bias[o] * S[t],
    S[t] = sum_{e,c} comb[t,e,c]
which cuts FLOPs ~3.4x and never materializes y (B,E,C,O).

Sharding: 8 cores; core k handles batch b=k//2 and expert group
h=k%2 (experts h*4..h*4+4) over ALL 4096 tokens. Each core produces a
partial out (T, O) summed over its 4 experts only; the host adds the
two partials of each batch pair (plus bias*S). This halves both the
dispatch-mask DMA and the dispatch matmul work per core versus
splitting tokens (where dispatch must be duplicated across the pair).

Dispatch runs xh-stationary: one 128x128 ldweights per token tile and
a 512-wide mask stream, producing xd^T[j,c] in PSUM; 16 PE transposes
recover xd[c,j] for the z stage. All matmuls are bf16 with fp32 PSUM
accumulation; partial outputs are stored bf16 (host sums in fp32).
"""

import numpy as np
import ml_dtypes

BF16 = ml_dtypes.bfloat16

B, T, E, C = 4, 4096, 8, 512
I = 128            # per-expert input features
O = 4096           # out_features
NCORES = 8
EL = E // 2        # experts per core
NTT = T // 128     # 32 token tiles
NCT = C // 128     # 4 c-tiles
NTCH = T // 512    # 8 t-chunks (z / combine granularity)
NOT2 = O // 1024   # 4 o-tiles (1024-wide moving operand in combine)

_CACHE = {}


def _build():
    import concourse.bass as bass
    import concourse.tile as tile
    import concourse.bacc as bacc
    import concourse.mybir as mybir

    f32 = mybir.dt.float32
    bf16 = mybir.dt.bfloat16
    ts = bass.ts

    nc = bacc.Bacc(None, target_bir_lowering=False, debug=False)

    xh = nc.dram_tensor("xh", [EL, 128, NTT, I], bf16, kind="ExternalInput")
    mh = nc.dram_tensor("mh", [EL, 128, NTT, C], bf16, kind="ExternalInput")
    cbt = nc.dram_tensor("cbt", [EL, NCT, 128, T], bf16, kind="ExternalInput")
    wf = nc.dram_tensor("wf", [NOT2, 128, EL, 1024], bf16,
                        kind="ExternalInput")
    ident = nc.dram_tensor("ident", [128, 128], bf16, kind="ExternalInput")
    out_d = nc.dram_tensor("out", [T, O], bf16, kind="ExternalOutput")

    with tile.TileContext(nc) as tc:
        with (
            tc.tile_pool(name="persist", bufs=1) as persist,
            tc.tile_pool(name="psumB", bufs=1, space="PSUM") as psumb,
        ):
            wf_sb = persist.tile([128, EL, O], bf16, tag="wf")
            id_sb = persist.tile([128, 128], bf16, tag="ident")
            nc.scalar.dma_start(id_sb[:], ident[:])

            xd = {}   # e -> xd tile [128c, (ct j)] bf16
            zt = {}   # (e, tch) -> z^T tile [128j, 512t] bf16

            def stage_b(e, tch, cb_pool, cb_bufs):
                # z^T[e][tch][j, t] = sum_c xd[e][c, j] * comb^T[c, t]
                cb_t = cb_pool.tile([128, NCT, 512], bf16, tag="cb",
                                    bufs=cb_bufs, name=f"cb{e}_{tch}")
                for ct in range(NCT):
                    nc.sync.dma_start(cb_t[:, ct, :],
                                      cbt[e, ct, :, ts(tch, 512)])
                ps_b = psumb.tile([128, 512], f32, tag="psB", bufs=2,
                                  name=f"psB{e}_{tch}")
                for ct in range(NCT):
                    nc.tensor.matmul(
                        ps_b[:],
                        xd[e][:, ts(ct, 128)],
                        cb_t[:, ct, :],
                        start=(ct == 0),
                        stop=(ct == NCT - 1),
                    )
                z_sb = persist.tile([128, 512], bf16, tag=f"zt{e}_{tch}",
                                    name=f"zt{e}_{tch}")
                nc.vector.tensor_copy(z_sb[:], ps_b[:])
                zt[(e, tch)] = z_sb

            # ---- Head phase: dispatch (DMA-bound on the mask stream) ----
            with (
                tc.tile_pool(name="head", bufs=1) as head,
                tc.tile_pool(name="psumD", bufs=1, space="PSUM") as psumd,
            ):
                for e in range(EL):
                    xh_t = head.tile([128, NTT, I], bf16, tag="xh", bufs=2,
                                     name=f"xh{e}")
                    nc.scalar.dma_start(xh_t[:], xh[e])
                    # xd^T accumulator: [128j, 512c], one chain over all tt
                    ps_d = psumd.tile([128, C], f32, tag="psD", bufs=2,
                                      name=f"psD{e}")
                    for q in range(NTT // 8):
                        mh_t = head.tile([128, 8, C], bf16, tag="mh", bufs=5,
                                         name=f"mh{e}_{q}")
                        nc.sync.dma_start(mh_t[:], mh[e, :, q * 8:q * 8 + 8, :])
                        for i in range(8):
                            tt = q * 8 + i
                            nc.tensor.matmul(
                                ps_d[:],
                                xh_t[:, tt, :],
                                mh_t[:, i, :],
                                start=(tt == 0),
                                stop=(tt == NTT - 1),
                            )
                    xdT_sb = head.tile([128, C], bf16, tag="xdT", bufs=2,
                                       name=f"xdT{e}")
                    nc.vector.tensor_copy(xdT_sb[:], ps_d[:])
                    xd_sb = persist.tile([128, C], bf16, tag=f"xd{e}",
                                         name=f"xd{e}")
                    for ct in range(NCT):
                        ps_t = psumd.tile([128, 128], bf16, tag="psT", bufs=2,
                                          name=f"psT{e}_{ct}")
                        nc.tensor.transpose(ps_t[:], xdT_sb[:, ts(ct, 128)],
                                            id_sb[:])
                        nc.scalar.copy(xd_sb[:, ts(ct, 128)], ps_t[:])
                    xd[e] = xd_sb
                    stage_b(e, 0, head, 2)
                    # weight o-slices: first half must be resident at tail
                    # start; second half rides just behind the head loads
                    # (first used a few final chains into the tail).
                    if e == 1:
                        for ot in (0, 1):
                            nc.scalar.dma_start(wf_sb[:, :, ts(ot, 1024)],
                                                wf[ot])
                    elif e == EL - 1:
                        for ot in (2, 3):
                            nc.sync.dma_start(wf_sb[:, :, ts(ot, 1024)],
                                              wf[ot])

            # ---- Combine phase (PE-bound), stage B interleaved per tch ----
            with (
                tc.tile_pool(name="tail", bufs=1) as tail,
                tc.tile_pool(name="psumC", bufs=1, space="PSUM") as psumc,
            ):
                for tcg in range(NTCH):
                    for tt in range(tcg * 4, tcg * 4 + 4):
                        m = tt % 4
                        out_sb = tail.tile([128, O], bf16, tag="out",
                                           bufs=3, name=f"out{tt}")
                        for ot in range(NOT2 * 2):
                            ps_c = psumc.tile([128, 512], f32, tag="psC",
                                              bufs=5, name=f"psC{tt}_{ot}")
                            for e in range(EL):
                                nc.tensor.matmul(
                                    ps_c[:],
                                    zt[(e, tcg)][:, ts(m, 128)],
                                    wf_sb[:, e, ts(ot, 512)],
                                    start=(e == 0),
                                    stop=(e == EL - 1),
                                )
                            if ot % 2 == 0:
                                nc.vector.tensor_copy(
                                    out_sb[:, ts(ot, 512)], ps_c[:])
                            else:
                                nc.scalar.copy(
                                    out_sb[:, ts(ot, 512)], ps_c[:])
                        nc.scalar.dma_start(out_d[ts(tt, 128), :], out_sb[:])
                    if tcg + 1 < NTCH:
                        for e in range(EL):
                            stage_b(e, tcg + 1, tail, 6)

    nc.compile()
    return nc


def _prep_inputs(x, weight, bias, combine_array, dispatch_mask):
    """Host-side cast to bf16 + re-layout for contiguous device DMA."""
    x = np.asarray(x, np.float32)
    weight = np.asarray(weight, np.float32)
    bias = np.asarray(bias, np.float32)
    comb = np.asarray(combine_array, np.float32)
    mask = np.asarray(dispatch_mask, np.float32)

    # xh: (B, E, 128, NTT, I); xh[b, e, p, tt, j] = x[b, tt*128+p, e, j]
    xh = np.ascontiguousarray(
        x.reshape(B, NTT, 128, E, I).transpose(0, 3, 2, 1, 4)).astype(BF16)
    # mh: (B, E, 128, NTT, C)
    mh = np.ascontiguousarray(
        mask.reshape(B, NTT, 128, E, C).transpose(0, 3, 2, 1, 4)).astype(BF16)
    # cbt: (B, E, NCT, 128, T); [..., e, ct, p, t] = comb[b, t, e, ct*128+p]
    cbt = np.ascontiguousarray(
        comb.reshape(B, T, E, NCT, 128).transpose(0, 2, 3, 4, 1)).astype(BF16)
    # wf: (NOT2, 128, E, 1024); wf[ot, j, e, oq] =
    #     weight.reshape(E, O, I)[e, ot*1024+oq, j]
    wf = np.ascontiguousarray(
        weight.reshape(E, NOT2, 1024, I).transpose(1, 3, 0, 2)).astype(BF16)
    # S[b, t] = sum_{e,c} comb[b, t, e, c] -- bias*S added on host in f32
    s = comb.sum(axis=(2, 3))
    idm = np.eye(128, dtype=BF16)

    in_maps = []
    for k in range(NCORES):
        b, h = k // 2, k % 2
        es = slice(h * EL, (h + 1) * EL)
        in_maps.append({
            "xh": np.ascontiguousarray(xh[b, es]),
            "mh": np.ascontiguousarray(mh[b, es]),
            "cbt": np.ascontiguousarray(cbt[b, es]),
            "wf": np.ascontiguousarray(wf[:, :, es, :]),
            "ident": idm,
        })
    return in_maps, s, bias


def kernel(x, weight, bias, combine_array, dispatch_mask):
    from concourse import bass_utils

    if "nc" not in _CACHE:
        _CACHE["nc"] = _build()
    nc = _CACHE["nc"]

    in_maps, s, bias_f = _prep_inputs(
        x, weight, bias, combine_array, dispatch_mask)
    res = bass_utils.run_bass_kernel_spmd(
        nc, in_maps, core_ids=list(range(NCORES)))
    out = np.empty((B, T, O), np.float32)
    for b in range(B):
        out[b] = res.results[2 * b]["out"].astype(np.float32)
        out[b] += res.results[2 * b + 1]["out"].astype(np.float32)
    out += s[:, :, None] * bias_f[None, None, :]
    return out
